# revision 1
# baseline (speedup 1.0000x reference)
"""Trainium2 Bass kernel for nn_ADDSEDiT_4097398800751 (8-layer AdaLN DiT).

Sharding: 8 cores = 4 batches x 2 sequence halves. Per-core activations are
kept transposed: [C on partitions (8 tiles), 512 tokens free]. Per-layer
cross-core comm: K^T/V halves exchanged within core pairs. AdaLN modulation
is channel-sharded across each pair and pair-AllGathered once at start.

Self-contained: hardcodes all shapes; builds in_maps internally.
"""
import contextlib
import numpy as np

import concourse.bass as bass
import concourse.mybir as mybir
import concourse.tile as tile
from concourse import bacc
from concourse import bass_utils
from concourse.dve_ops import AFFINE_THEN_ADD

FP = mybir.dt.float32
FR = mybir.dt.float32r
BF = mybir.dt.bfloat16
AF = mybir.ActivationFunctionType
ALU = mybir.AluOpType

C = 1024
H = 16
D = 64
NL = 8
B = 4
L = 1024
S = 512          # tokens per core
CT = C // 128    # 8 channel tiles
NT = S // 128    # 4 token tiles
SKIP = 2 ** -0.5
EPS = 1e-6
SCALE = D ** -0.5

_graph_cache = {}


def build_graph(n_layers=NL, kv_mode="ag"):
    nc = bacc.Bacc("TRN2", target_bir_lowering=False, debug=False, num_devices=8)

    def din(name, shape, dt=FR):
        return nc.dram_tensor(name, list(shape), dt, kind="ExternalInput").ap()

    T = dict(
        xT=din("xT", [CT, 128, S]),
        cosT2=din("cosT2", [128, S], BF),
        s1T2=din("s1T2", [128, S], BF),
        pmat=din("pmat", [128, 128], BF),
        ones=din("ones", [128, 128]),
        qkv_w=din("qkv_w", [n_layers, CT, 128, 3 * C], BF),
        proj_w=din("proj_w", [n_layers, CT, 128, C], BF),
        mlp_w1=din("mlp_w1", [n_layers, CT, 128, 4 * C], BF),
        mlp_w2=din("mlp_w2", [n_layers, 4 * CT, 128, C], BF),
        adaln_w=din("adaln_w", [n_layers, CT, 128, 3072], BF),
        cmodT=din("cmodT", [CT, 128, 1], FP),
        bias_qk=din("bias_qk", [n_layers, 16, 128], FP),
        bias_v=din("bias_v", [n_layers, 16, 64], FP),
        bias_pj=din("bias_pj", [n_layers, CT, 128], FP),
        bias_m1=din("bias_m1", [n_layers, 4 * CT, 128], FP),
        bias_m2=din("bias_m2", [n_layers, CT, 128], FP),
        adaln_b=din("adaln_b", [2 * n_layers * 24, 128], FP),
        out=nc.dram_tensor("out", [CT, 128, S], FP, kind="ExternalOutput").ap(),
    )

    with tile.TileContext(nc) as tc:
        with contextlib.ExitStack() as ctx:
            with nc.allow_low_precision(reason="f32r compute by design"):
                _build_body(nc, tc, n_layers, kv_mode, T, ctx)
    nc.compile()
    return nc


def _build_body(nc, tc, n_layers, kv_mode, T, ctx):
    pers = ctx.enter_context(tc.tile_pool(name="pers", bufs=1))
    wpool = ctx.enter_context(tc.tile_pool(name="w", bufs=3))
    tmp = ctx.enter_context(tc.tile_pool(name="tmp", bufs=3))
    ps2 = ctx.enter_context(tc.tile_pool(name="ps2", bufs=2, space="PSUM"))
    dram = ctx.enter_context(tc.tile_pool(name="dram", bufs=1, space="DRAM"))

    # ---- constants ----
    ones_f = pers.tile([128, 128], FR, name="ones_f")
    nc.sync.dma_start(ones_f[:], T["ones"])
    onecol = ones_f[:, 0:1]          # [128,1] f32r
    onerow = ones_f[0:1, :]          # [1,128] f32r
    eps_sb = pers.tile([128, 1], FP, name="eps_sb")
    nc.vector.memset(eps_sb[:], EPS)

    # ---- persistent tensors ----
    y = pers.tile([128, CT, S], FR, name="y")
    cos_sb = pers.tile([128, S], BF, name="cos_sb")
    s1_sb = pers.tile([128, S], BF, name="s1_sb")
    pm_sb = pers.tile([128, 128], BF, name="pm_sb")
    nc.sync.dma_start(y[:], T["xT"].rearrange("t p s -> p t s"))
    nc.sync.dma_start(cos_sb[:], T["cosT2"])
    nc.sync.dma_start(s1_sb[:], T["s1T2"])
    nc.sync.dma_start(pm_sb[:], T["pmat"])

    # per-layer vectors: slots 0..7 = sc1,sh1,g1,gb1,sc2,sh2,g2,gb2
    vec = pers.tile([128, n_layers, 8, CT], FP, name="vec")
    b_qk = pers.tile([128, n_layers, 16], FP, name="b_qk")
    b_v = pers.tile([64, n_layers, 16], FP, name="b_v")
    b_pj = pers.tile([128, n_layers, CT], FP, name="b_pj")
    b_m1 = pers.tile([128, n_layers, 4 * CT], FP, name="b_m1")
    b_m2 = pers.tile([128, n_layers, CT], FP, name="b_m2")
    nc.sync.dma_start(b_qk[:], T["bias_qk"].rearrange("l m p -> p l m"))
    nc.sync.dma_start(b_v[:], T["bias_v"].rearrange("l m p -> p l m"))
    nc.sync.dma_start(b_pj[:], T["bias_pj"].rearrange("l m p -> p l m"))
    nc.sync.dma_start(b_m1[:], T["bias_m1"].rearrange("l m p -> p l m"))
    nc.sync.dma_start(b_m2[:], T["bias_m2"].rearrange("l m p -> p l m"))

    # =======================  AdaLN stage  =======================
    cm = pers.tile([128, CT, 1], FP, name="cm")
    nc.sync.dma_start(cm[:], T["cmodT"].rearrange("t p one -> p t one"))
    silu = pers.tile([128, CT, 1], BF, name="silu")
    nc.scalar.activation(silu[:], cm[:], AF.Silu)

    agin = dram.tile([1, n_layers, 3072], FP)
    agout = dram.tile([2, n_layers, 3072], FP)
    for l in range(n_layers):
        for c in range(3):
            mps = [ps2.tile([1, 512], FP, name="qp", tag="qp")
                   for _ in range(2)]
            for k in range(CT):
                aw = wpool.tile([128, 1024], BF, name=f"ws{k}", tag=f"ws{k}")
                nc.sync.dma_start(aw[:],
                                  T["adaln_w"][l, k, :, c * C:(c + 1) * C])
                for i in range(2):
                    nc.tensor.matmul(mps[i][:], silu[:, k, :],
                                     aw[:, i * 512:(i + 1) * 512],
                                     start=(k == 0), stop=(k == CT - 1))
            for i in range(2):
                n = 2 * c + i
                mrow = tmp.tile([1, 512], FP, name="mrow", tag="mrow", bufs=2)
                nc.scalar.copy(mrow[:], mps[i][:])
                nc.sync.dma_start(agin[:, l, n * 512:(n + 1) * 512], mrow[:])
    nc.gpsimd.collective_compute(
        "AllGather", ALU.bypass,
        ins=[agin[:].opt()], outs=[agout[:].opt()],
        replica_groups=[[0, 1], [2, 3], [4, 5], [6, 7]],
    )
    # modv[p, g, l, v, t]: channel 128*(g*4+t)+p of vec v
    modv = pers.tile([128, 2, n_layers, 6, NT], FP, name="modv")
    for g in range(2):
        nc.sync.dma_start(
            modv[:, g], agout[g].rearrange("l (m p) -> p l m", p=128))
    absb = pers.tile([128, 2, n_layers, 6, NT], FP, name="absb")
    nc.sync.dma_start(absb[:], T["adaln_b"].rearrange("n p -> p n"))
    nc.vector.tensor_tensor(modv[:], modv[:], absb[:], ALU.add)

    for l in range(n_layers):
        sk1 = float(SKIP ** -(2 * l))
        sk2 = float(SKIP ** -(2 * l + 1))
        nc.vector.tensor_scalar(vec[:, l, 0, :].rearrange("p (g t) -> p g t", g=2), modv[:, :, l, 1, :],
                                float(SKIP), float(SKIP), ALU.mult, ALU.add)
        nc.vector.tensor_scalar(vec[:, l, 1, :].rearrange("p (g t) -> p g t", g=2), modv[:, :, l, 0, :],
                                float(SKIP), None, ALU.mult)
        nc.vector.tensor_scalar(vec[:, l, 2, :].rearrange("p (g t) -> p g t", g=2), modv[:, :, l, 2, :],
                                sk1, None, ALU.mult)
        nc.vector.tensor_tensor(vec[:, l, 3, :], vec[:, l, 2, :],
                                b_pj[:, l, :], ALU.mult)
        nc.vector.tensor_scalar(vec[:, l, 4, :].rearrange("p (g t) -> p g t", g=2), modv[:, :, l, 4, :],
                                float(SKIP), float(SKIP), ALU.mult, ALU.add)
        nc.vector.tensor_scalar(vec[:, l, 5, :].rearrange("p (g t) -> p g t", g=2), modv[:, :, l, 3, :],
                                float(SKIP), None, ALU.mult)
        nc.vector.tensor_scalar(vec[:, l, 6, :].rearrange("p (g t) -> p g t", g=2), modv[:, :, l, 5, :],
                                sk2, None, ALU.mult)
        nc.vector.tensor_tensor(vec[:, l, 7, :], vec[:, l, 6, :],
                                b_m2[:, l, :], ALU.mult)

    # =======================  main stack  =======================
    h = pers.tile([128, CT, S], BF, name="h")
    stat = pers.tile([1, 3, S], FP, name="stat")
    statr = pers.tile([1, 2, S], FR, name="statr")
    rec = pers.tile([128, 2, 512], FR, name="rec")

    kv_in = dram.tile([2, CT, 128, S], mybir.dt.bfloat16)
    kv_out = dram.tile([2, 2, CT, 128, S], mybir.dt.bfloat16)

    def wslabs(src_aps):
        slabs = []
        for k in range(CT):
            wt = wpool.tile([128, 1024], BF, name=f"ws{k}", tag=f"ws{k}")
            nc.sync.dma_start(wt[:], src_aps[k])
            slabs.append(wt)
        return slabs

    def layernorm(l, vslot):
        ssum = ps2.tile([1, 512], FP, name="ssum", tag="op", bufs=1)
        ssq = ps2.tile([1, 512], FP, name="ssq", tag="dpbc", bufs=1)
        for t in range(CT):
            sq = tmp.tile([128, S], FR, name="rt", tag="rt", bufs=2)
            nc.vector.tensor_tensor(sq[:], y[:, t, :], y[:, t, :], ALU.mult)
            nc.tensor.matmul(ssum[:], onecol, y[:, t, :],
                             start=(t == 0), stop=(t == CT - 1))
            nc.tensor.matmul(ssq[:], onecol, sq[:],
                             start=(t == 0), stop=(t == CT - 1))
        # stat: 0=mu, 1=var, 2=tmp ; statr: 0=A(rstd), 1=mu*A
        nc.vector.tensor_scalar(stat[:, 0, :], ssum[:], 1.0 / C, None, ALU.mult)
        nc.vector.tensor_scalar(stat[:, 1, :], ssq[:], 1.0 / C, None, ALU.mult)
        nc.vector.tensor_tensor(stat[:, 2, :], stat[:, 0, :], stat[:, 0, :],
                                ALU.mult)
        nc.vector.tensor_tensor(stat[:, 1, :], stat[:, 1, :], stat[:, 2, :],
                                ALU.subtract)
        nc.scalar.activation(stat[:, 2, :], stat[:, 1, :], AF.Ln,
                             bias=eps_sb[0:1, :])
        nc.scalar.activation(statr[:, 0, :], stat[:, 2, :],
                             AF.Exp, scale=-0.5)
        nc.vector.tensor_tensor(statr[:, 1, :], stat[:, 0, :],
                                statr[:, 0, :], ALU.mult)
        bcA = ps2.tile([128, 512], FP, name="bcA", tag="op", bufs=1)
        bcB = ps2.tile([128, 512], FP, name="bcB", tag="dpbc", bufs=1)
        nc.tensor.matmul(bcA[:], onerow, statr[:, 0, :],
                         start=True, stop=True)
        nc.tensor.matmul(bcB[:], onerow, statr[:, 1, :],
                         start=True, stop=True)
        for t in range(CT):
            u = tmp.tile([128, S], FP, name="qc", tag="qc", bufs=2)
            nc.vector.tensor_tensor(u[:], y[:, t, :], bcA[:], ALU.mult)
            nc.vector.tensor_tensor(u[:], u[:], bcB[:], ALU.subtract)
            nc.scalar.activation(h[:, t, :], u[:], AF.Identity,
                                 bias=vec[:, l, vslot + 1, t:t + 1],
                                 scale=vec[:, l, vslot, t:t + 1])

    def qkv_kv(l):
        kT = tmp.tile([128, CT, S], BF, name="kT_expa", tag="kT_expa", bufs=2)
        v_sb = tmp.tile([128, NT, C], BF, name="v_expb", tag="v_expb", bufs=2)
        ws = wslabs([T["qkv_w"][l, k, :, C:2 * C] for k in range(CT)])
        for m in range(8):
            qp = ps2.tile([128, 512], FP, name="qp")
            for k in range(CT):
                nc.tensor.matmul(qp[:], ws[k][:, m * 128:(m + 1) * 128],
                                 h[:, k, :], start=(k == 0),
                                 stop=(k == CT - 1))
            nc.vector.tensor_scalar(kT[:, m, :], qp[:],
                                    b_qk[:, l, 8 + m:9 + m], None, ALU.add)
            rope(kT[:, m, :], m)
        ws = wslabs([T["qkv_w"][l, k, :, 2 * C:3 * C] for k in range(CT)])
        for mt in range(NT):
            for n in range(2):
                vp = ps2.tile([128, 512], FP, name="qp")
                for k in range(CT):
                    nc.tensor.matmul(vp[:],
                                     h[:, k, mt * 128:(mt + 1) * 128],
                                     ws[k][:, n * 512:(n + 1) * 512],
                                     start=(k == 0), stop=(k == CT - 1))
                nc.vector.tensor_copy(v_sb[:, mt, n * 512:(n + 1) * 512],
                                      vp[:])
        return kT, v_sb

    def qkv_q(l):
        qT = []
        ws = wslabs([T["qkv_w"][l, k, :, 0:C] for k in range(CT)])
        for m in range(8):
            qTj = tmp.tile([128, S], BF, name=f"oq{m}", tag=f"oq{m}", bufs=2)
            qp = ps2.tile([128, 512], FP, name="qp")
            for k in range(CT):
                nc.tensor.matmul(qp[:], ws[k][:, m * 128:(m + 1) * 128],
                                 h[:, k, :], start=(k == 0),
                                 stop=(k == CT - 1))
            nc.vector.tensor_scalar(qTj[:], qp[:], b_qk[:, l, m:m + 1],
                                    None, ALU.add)
            rope(qTj[:], m)
            qT.append(qTj)
        return qT

    def rope(ap, idx):
        """In-place RoPE on [128, S] tile (2 heads, planar layout).
        rotate-half (with signs) via a PE permutation matmul."""
        tag = "op" if idx % 2 == 0 else "dpbc"
        sw = ps2.tile([128, 512], FP, name="sw", tag=tag, bufs=1)
        nc.tensor.matmul(sw[:], pm_sb[:], ap, start=True, stop=True)
        rt = tmp.tile([128, S], BF, name="rt", tag="rt", bufs=2)
        nc.vector.tensor_tensor(rt[:], ap, cos_sb[:], ALU.mult)
        qc = tmp.tile([128, S], BF, name="qc", tag="qc", bufs=2)
        nc.vector.tensor_tensor(qc[:], sw[:], s1_sb[:], ALU.mult)
        nc.vector.tensor_tensor(ap, rt[:], qc[:], ALU.add)

    def kv_exchange(l, kT, v_sb):
        nc.sync.dma_start(kv_in[0].rearrange("t p s -> p t s"), kT[:])
        nc.sync.dma_start(
            kv_in[1].rearrange("(mt a) p s -> p mt a s", a=2),
            v_sb[:].rearrange("p mt (a s) -> p mt a s", a=2))
        nc.gpsimd.collective_compute(
            "AllGather", ALU.bypass,
            ins=[kv_in[:].opt()], outs=[kv_out[:].opt()],
            replica_groups=[[0, 1], [2, 3], [4, 5], [6, 7]],
        )

    def attention(l, qT):
        oT = []
        for j in range(8):
            kTj = tmp.tile([128, L], BF, name="kTj", tag="kTj", bufs=2)
            vj = tmp.tile([128, CT, 130], BF, name="vj", tag="vj", bufs=2)
            for r in range(2):
                nc.sync.dma_start(kTj[:, r * S:(r + 1) * S],
                                  kv_out[r, 0, j])
                a, so = j // 4, 128 * (j % 4)
                for hh in range(2):
                    nc.sync.dma_start(
                        vj[:, r * NT:(r + 1) * NT, 65 * hh:65 * hh + 64],
                        kv_out[r, 1, a::2, :,
                               so + 64 * hh:so + 64 * hh + 64].rearrange(
                            "mt p d -> p mt d"))
            nc.vector.tensor_copy(
                vj[:, :, 64:130:65],
                ones_f[:, 0:16].rearrange("p (a b) -> p a b", b=2))
            expa = tmp.tile([128, CT, S], BF, name="kT_expa", tag="kT_expa",
                            bufs=2)
            expb = tmp.tile([128, CT, S], BF, name="v_expb", tag="v_expb",
                            bufs=2)
            for mg in range(4):
                sca = ps2.tile([128, 2, 512], FP, name="sca", tag="sca",
                               bufs=1)
                scb = ps2.tile([128, 2, 512], FP, name="scb", tag="scb",
                               bufs=1)
                for mi in range(2):
                    m = mg * 2 + mi
                    for hh in range(2):
                        b = 64 * hh
                        dst = (sca if hh == 0 else scb)
                        nc.tensor.matmul(
                            dst[:, mi, :], kTj[b:b + 64, m * 128:(m + 1) * 128],
                            qT[j][b:b + 64, :], start=True, stop=True,
                            tile_position=(b, 0))
                nc.scalar.activation(expa[:, mg * 2:mg * 2 + 2, :], sca[:],
                                     AF.Exp, scale=SCALE)
                nc.scalar.activation(expb[:, mg * 2:mg * 2 + 2, :], scb[:],
                                     AF.Exp, scale=SCALE)
            opa = ps2.tile([128, 512], FP, name="opa", tag="op", bufs=1)
            opb = ps2.tile([128, 512], FP, name="opb", tag="dpbc", bufs=1)
            for kt in range(CT):
                nc.tensor.matmul(opa[0:65, :], vj[:, kt, 0:65],
                                 expa[:, kt, :], start=(kt == 0),
                                 stop=(kt == CT - 1))
                nc.tensor.matmul(opb[0:65, :], vj[:, kt, 65:130],
                                 expb[:, kt, :], start=(kt == 0),
                                 stop=(kt == CT - 1))
            nc.vector.reciprocal(rec[64:65, 0, :], opa[64:65, :])
            nc.vector.reciprocal(rec[64:65, 1, :], opb[64:65, :])
            bca = ps2.tile([64, 512], FP, name="bca", tag="sca", bufs=1)
            bcb = ps2.tile([64, 512], FP, name="bcb", tag="scb", bufs=1)
            nc.tensor.matmul(bca[:], ones_f[64:65, 0:64], rec[64:65, 0, :],
                             start=True, stop=True, tile_position=(64, 0))
            nc.tensor.matmul(bcb[:], ones_f[64:65, 0:64], rec[64:65, 1, :],
                             start=True, stop=True, tile_position=(64, 0))
            oTj = tmp.tile([128, S], BF, name=f"oq{j}", tag=f"oq{j}", bufs=2)
            bcs = tmp.tile([64, S], FP, name="rt", tag="rt", bufs=2)
            nc.vector.tensor_copy(bcs[:], bca[:])
            nc.vector.tensor_tensor(oTj[0:64, :], opa[0:64, :],
                                    bcs[:], ALU.mult)
            nc.vector.tensor_scalar(oTj[0:64, :], oTj[0:64, :],
                                    b_v[:, l, 2 * j:2 * j + 1], None, ALU.add)
            bcs2 = tmp.tile([64, S], FP, name="rt", tag="rt", bufs=2)
            nc.vector.tensor_copy(bcs2[:], bcb[:])
            tmpB = tmp.tile([64, S], BF, name="qc", tag="qc", bufs=2)
            nc.vector.tensor_tensor(tmpB[:], opb[0:64, :], bcs2[:], ALU.mult)
            nc.vector.tensor_scalar(tmpB[:], tmpB[:],
                                    b_v[:, l, 2 * j + 1:2 * j + 2],
                                    None, ALU.add)
            nc.sync.dma_start(oTj[64:128, :], tmpB[:])
            oT.append(oTj)
        return oT

    def proj(l, oT):
        ws = wslabs([T["proj_w"][l, k] for k in range(CT)])
        for m in range(CT):
            pp = ps2.tile([128, 512], FP, name="qp", tag="qp")
            for k in range(CT):
                nc.tensor.matmul(pp[:], ws[k][:, m * 128:(m + 1) * 128],
                                 oT[k][:], start=(k == 0), stop=(k == CT - 1))
            nc.vector._custom_dve(
                AFFINE_THEN_ADD, out=y[:, m, :], in0=pp[:], in1=y[:, m, :],
                s0=vec[:, l, 2, m:m + 1], s1=vec[:, l, 3, m:m + 1])

    def mlp(l):
        m2sb = tmp.tile([128, CT, S], FP, name="v_expb", tag="v_expb", bufs=2)
        for kg in range(4):
            hid = [tmp.tile([128, S], BF, name=f"hid{k}", tag=f"oq{k}",
                            bufs=2) for k in range(CT)]
            w1 = wslabs([T["mlp_w1"][l, k, :, kg * C:(kg + 1) * C]
                         for k in range(CT)])
            for m in range(8):
                mm = kg * 8 + m
                hp = ps2.tile([128, 512], FP, name="qp")
                for k in range(CT):
                    nc.tensor.matmul(hp[:], w1[k][:, m * 128:(m + 1) * 128],
                                     h[:, k, :], start=(k == 0),
                                     stop=(k == CT - 1))
                nc.scalar.activation(hid[m][:], hp[:], AF.Gelu_apprx_tanh,
                                     bias=b_m1[:, l, mm:mm + 1])
            w2 = wslabs([T["mlp_w2"][l, kg * 8 + k] for k in range(CT)])
            for m in range(CT):
                qp = ps2.tile([128, 512], FP, name="qp")
                for k in range(CT):
                    nc.tensor.matmul(qp[:], w2[k][:, m * 128:(m + 1) * 128],
                                     hid[k][:], start=(k == 0),
                                     stop=(k == CT - 1))
                if kg == 0:
                    nc.vector.tensor_copy(m2sb[:, m, :], qp[:])
                else:
                    nc.vector.tensor_tensor(m2sb[:, m, :], m2sb[:, m, :],
                                            qp[:], ALU.add)
        for m in range(CT):
            nc.vector._custom_dve(
                AFFINE_THEN_ADD, out=y[:, m, :], in0=m2sb[:, m, :],
                in1=y[:, m, :],
                s0=vec[:, l, 6, m:m + 1], s1=vec[:, l, 7, m:m + 1])

    for l in range(n_layers):
        layernorm(l, 0)
        kT, v_sb = qkv_kv(l)
        kv_exchange(l, kT, v_sb)
        qT = qkv_q(l)
        oT = attention(l, qT)
        proj(l, oT)
        layernorm(l, 4)
        mlp(l)

    # final scale + output (reuse h's slot; h is dead after the last MLP)
    osb = tmp.tile([128, CT, S], FP, name="kT_expa", tag="kT_expa", bufs=2)
    fs = float(SKIP ** (2 * n_layers))
    for t in range(CT):
        nc.scalar.activation(osb[:, t, :], y[:, t, :], AF.Copy, scale=fs)
    nc.sync.dma_start(T["out"].rearrange("t p s -> p t s"), osb[:])


# =====================  host side  =====================

def planar_perm():
    return np.concatenate([np.arange(0, D, 2), np.arange(1, D, 2)])


def host_prep(inputs, n_layers=NL):
    f32 = np.float32
    x = np.asarray(inputs['x'], f32)
    c = np.asarray(inputs['c'], f32)
    t = np.asarray(inputs['t'], f32)
    cos = np.asarray(inputs['cos'], f32)[:L]
    sin = np.asarray(inputs['sin'], f32)[:L]
    qkv_w = np.asarray(inputs['qkv_w'], f32)[:n_layers]
    qkv_b = np.asarray(inputs['qkv_b'], f32)[:n_layers]
    proj_w = np.asarray(inputs['proj_w'], f32)[:n_layers]
    proj_b = np.asarray(inputs['proj_b'], f32)[:n_layers]
    mlp_w1 = np.asarray(inputs['mlp_w1'], f32)[:n_layers]
    mlp_b1 = np.asarray(inputs['mlp_b1'], f32)[:n_layers]
    mlp_w2 = np.asarray(inputs['mlp_w2'], f32)[:n_layers]
    mlp_b2 = np.asarray(inputs['mlp_b2'], f32)[:n_layers]
    adaln_w = np.asarray(inputs['adaln_w'], f32)[:n_layers]
    adaln_b = np.asarray(inputs['adaln_b'], f32)[:n_layers]

    perm = planar_perm()
    qkv_w_p = qkv_w.copy()
    qkv_b_p = qkv_b.copy()
    for hh in range(H):
        for blk in range(2):
            cb = blk * C + hh * D
            qkv_w_p[:, :, cb:cb + D] = qkv_w[:, :, cb + perm]
            qkv_b_p[:, cb:cb + D] = qkv_b[:, cb + perm]

    cos_p = cos[:, perm]
    s1_p = sin[:, perm]
    pmat = np.zeros((128, 128), f32)
    for blk in range(2):
        bb = 64 * blk
        for d in range(32):
            pmat[bb + 32 + d, bb + d] = -1.0   # out[d] = -q[d+32]
            pmat[bb + d, bb + 32 + d] = 1.0    # out[d+32] = q[d]
    cmodT_all = ((c[:, 0, :] + t) * SKIP).T.astype(f32)   # [C, B]

    ca = np.ascontiguousarray

    def pack_bias(b2d):
        nl, cc = b2d.shape
        return ca(b2d.reshape(nl, cc // 128, 128))

    import ml_dtypes
    bf = ml_dtypes.bfloat16
    shared = dict(
        qkv_w=ca(qkv_w_p.reshape(n_layers, CT, 128, 3 * C).astype(bf)),
        proj_w=ca(proj_w.reshape(n_layers, CT, 128, C).astype(bf)),
        mlp_w1=ca(mlp_w1.reshape(n_layers, CT, 128, 4 * C).astype(bf)),
        mlp_w2=ca(mlp_w2.reshape(n_layers, 4 * CT, 128, C).astype(bf)),
        bias_qk=pack_bias(qkv_b_p[:, 0:2 * C]),
        bias_v=ca(qkv_b_p[:, 2 * C:3 * C].reshape(n_layers, 16, 64)),
        bias_pj=pack_bias(proj_b),
        bias_m1=pack_bias(mlp_b1),
        bias_m2=pack_bias(mlp_b2),
        adaln_b=ca(adaln_b.reshape(n_layers, 6, 2, 4, 128)
                   .transpose(2, 0, 1, 3, 4).reshape(2 * n_layers * 24, 128)),
    )

    in_maps = []
    for core in range(8):
        b, seg = core // 2, core % 2
        sl = slice(seg * S, (seg + 1) * S)
        ch = np.arange(seg * S, seg * S + S)
        cols = np.concatenate([v * C + ch for v in range(6)])
        m = dict(
            shared,
            xT=ca(x[b, sl, :].T.reshape(CT, 128, S)),
            cosT2=ca(np.vstack([cos_p[sl].T, cos_p[sl].T]).astype(bf)),
            s1T2=ca(np.vstack([s1_p[sl].T, s1_p[sl].T]).astype(bf)),
            pmat=pmat.astype(bf),
            ones=np.ones((128, 128), f32),
            adaln_w=ca(adaln_w[:, :, cols].reshape(
                n_layers, CT, 128, 3072).astype(bf)),
            cmodT=ca(cmodT_all[:, b:b + 1].reshape(CT, 128, 1)),
        )
        in_maps.append(m)
    return in_maps


def run(inputs, n_layers=NL, kv_mode="ag", trace=False):
    key = (n_layers, kv_mode)
    if key not in _graph_cache:
        _graph_cache[key] = build_graph(n_layers, kv_mode)
    nc = _graph_cache[key]
    in_maps = host_prep(inputs, n_layers)
    res = bass_utils.run_bass_kernel_spmd(
        nc, in_maps, core_ids=list(range(8)), trace=trace)
    outs = np.zeros((B, L, C), dtype=np.float32)
    for core in range(8):
        b, seg = core // 2, core % 2
        o = res.results[core]["out"].reshape(C, S)
        outs[b, seg * S:(seg + 1) * S, :] = o.T
    return outs, res


def kernel(**inputs) -> np.ndarray:
    out, _ = run(inputs, n_layers=NL)
    return out

